# revision 44
# baseline (speedup 1.0000x reference)
"""GATConv block (GAT attention + BatchNorm + leaky_relu) on 8 Trainium2 NeuronCores.

v2 strategy (graph/data parallel):
- Nodes sharded across 8 cores by destination range (12500 each).
- Phase 1 (replicated): each core computes the full feature table
  row[n] = [xt[n] (128) | a_src[n] (4) | pad (120)] in bf16, written as
  512B rows into 4 quarter-tables (so gather indices fit int16), plus a
  separate fp32 a_dst table.
- Phase 2: edges grouped by dst block (<=128 dst nodes/position), sorted by
  src within the block, split into 4 runs by src quarter.  Each run is ONE
  dma_gather (512B rows) on its own SWDGE queue (4 queues drain in parallel
  -- ~5x the single-queue indirect-DMA rate).  Per-edge a_dst comes from a
  one-hot lookup built on PE+DVE from host-provided dst offsets; attention
  weights exp(leaky(a_src+a_dst)) multiply xt in bf16 on DVE, and a per-tile
  PE matmul  out[dst_block] += S.T @ [w*xt | w]  accumulates numerator and
  softmax denominator in fp32 PSUM.
- Phase 3: BatchNorm batch stats via ones-vector matmuls, AllReduce across
  cores, per-channel affine + leaky_relu.

All data-dependent structure is host data; tile counts per (position, quarter)
are equalized across cores (position-wise sort + max padding), with pad gather
slots pointing at row 0 (gated out by rel=-1 one-hot misses).
"""

import sys

sys.path.insert(0, "/opt/trn_rl_repo")

import numpy as np
from contextlib import ExitStack

import concourse.bass as bass
import concourse.mybir as mybir
import concourse.tile as tile
from concourse import bacc

# --- queue-aware SWDGE semaphore-lane assignment -------------------------
# Tile round-robins Pool-engine DMA completion sems over 8 DMASW lanes with
# no regard for the SWDGE queue, but a sem may only be incremented from one
# queue.  Dedicate lanes {2q, 2q+1} to queue q (InstDMACopy et al. without a
# queue_num always execute on queue 0).
import concourse.tile_sem_assignment as _tsa
import concourse.bass_isa as _bass_isa

if not getattr(_tsa, "_qaware_patch", False):
    _tsa._qaware_patch = True
    _DMAInst = _tsa.DMAInst

    _orig_assign_tick = _tsa.TileClockTick._assign_tick

    def _assign_tick_qaware(self, inst):
        if (
            isinstance(inst, _DMAInst)
            and not isinstance(inst, _bass_isa.UserSyncedRemoteDMADescs)
            and inst.engine == mybir.EngineType.Pool
        ):
            qn = getattr(inst, "queue_num", 0) or 0
            rot = getattr(self, "_qaware_rot", None)
            if rot is None:
                rot = self._qaware_rot = {}
            r = rot.get(qn, 0)
            rot[qn] = r ^ 1
            self.next_sw_dma_idx = (2 * qn + r) % self.swdge_sem_count
        return _orig_assign_tick(self, inst)

    _tsa.TileClockTick._assign_tick = _assign_tick_qaware
# -------------------------------------------------------------------------

FP32 = mybir.dt.float32
BF16 = mybir.dt.bfloat16
I32 = mybir.dt.int32
I16 = mybir.dt.int16

N = 100000
E = 1600000
F_IN = 128
H = 4
C = 32
F_OUT = H * C
NEG = 0.2
EPS = 1e-5
NCORES = 8
P = 128
ROWC = 256          # table row: [xt 128 | a_src 4 | pad] bf16 = 512B
QS = 25088          # quarter size (multiple of 128, < 32768 for int16 idx)
QLO = [0, QS, 2 * QS, 3 * QS, N]
NQ = 4


class Cfg:
    def __init__(self, npos, tq, shard, ncores):
        self.npos = npos            # positions per core
        self.tq = tq                # [npos, NQ] tiles per (position, quarter)
        self.tlist = tq.sum(axis=1)  # [npos] total tiles per position
        self.shard = shard
        self.ncores = ncores
        self.TT = int(self.tlist.sum())
        self.offs = np.concatenate([[0], np.cumsum(self.tlist)]).astype(int)
        self.tmax = int(self.tlist.max())
        self.nxt = (N + P - 1) // P


def preprocess(edge_index, n=N, ncores=NCORES):
    src = np.asarray(edge_index[0]).astype(np.int64)
    dst = np.asarray(edge_index[1]).astype(np.int64)
    order = np.argsort(dst, kind="stable")
    s_src = src[order].astype(np.int32)
    s_dst = dst[order].astype(np.int32)

    shard = n // ncores
    npos = (shard + P - 1) // P

    # per core, per block: edge range + per-quarter run lengths
    blocks = []   # [c][b] = (nb_lo, n_nodes, e_lo, e_hi, qlens[4])
    for c_ in range(ncores):
        lo_n = c_ * shard
        hi_n = lo_n + shard
        bl = []
        for b in range(npos):
            nb_lo = lo_n + b * P
            nb_hi = min(nb_lo + P, hi_n)
            e_lo = int(np.searchsorted(s_dst, nb_lo, "left"))
            e_hi = int(np.searchsorted(s_dst, nb_hi, "left"))
            sseg = s_src[e_lo:e_hi]
            o2 = np.argsort(sseg, kind="stable")
            ss = sseg[o2]
            qb = [int(np.searchsorted(ss, QLO[q], "left")) for q in range(5)]
            qlens = [qb[q + 1] - qb[q] for q in range(4)]
            bl.append((nb_lo, nb_hi - nb_lo, e_lo, e_hi, o2, qlens))
        blocks.append(bl)

    tiles = np.zeros((ncores, npos), np.int64)
    tq_all = np.zeros((ncores, npos, NQ), np.int64)
    for c_ in range(ncores):
        for b in range(npos):
            ql = blocks[c_][b][5]
            for q in range(NQ):
                tq_all[c_, b, q] = -(-ql[q] // P)
            tiles[c_, b] = tq_all[c_, b].sum()

    # balance: per core sort blocks by total tiles desc, then per-(slot, q) max
    perm = np.argsort(-tiles, axis=1, kind="stable")
    tq_sorted = np.take_along_axis(
        tq_all, perm[:, :, None], axis=1)      # [c, g, q]
    tq_max = tq_sorted.max(axis=0)             # [g, q]
    # every position needs >=1 tile (pads are gated by rel=-1)
    empty = tq_max.sum(axis=1) == 0
    tq_max[empty, 0] = 1

    cfg = Cfg(npos, tq_max, shard, ncores)

    # host meta, per core:
    #  rel   [128, TT] int32 (fp32 bits; -1.0 for pads)
    #  idxw  [16, 8*TT] int16 (xt gather indices, 16-wrapped per run)
    #  idxad [16, 8*TT] int16 (a_dst gather indices = block*128+rel, wrapped
    #         per ADCH-tile chunk; 0 for pads)
    ADCH = 32
    rel = np.zeros((ncores, P, cfg.TT), np.int16)
    relT = np.zeros((ncores, 1, P * cfg.TT), np.int16)
    idxw = np.zeros((ncores, 16, 8 * cfg.TT), np.int16)
    idxad = np.zeros((ncores, 16, 8 * cfg.TT), np.int16)
    blkbase = np.zeros((ncores, P, 1), np.int32)
    out_map = []
    for c_ in range(ncores):
        omap = []
        for g in range(npos):
            b = int(perm[c_, g])
            nb_lo, n_nodes, e_lo, e_hi, o2, qlens = blocks[c_][b]
            sseg = s_src[e_lo:e_hi][o2]
            dseg = s_dst[e_lo:e_hi][o2]
            qb = np.concatenate([[0], np.cumsum(qlens)]).astype(int)
            toff0 = int(cfg.offs[g])
            toff = toff0
            rl_all = []
            for q in range(NQ):
                tqi = int(tq_max[g, q])
                if tqi == 0:
                    continue
                L = qlens[q]
                npad = tqi * P
                loc = np.zeros(npad, np.int32)
                rl = np.full(npad, -1.0, np.float32)
                loc[:L] = sseg[qb[q]:qb[q + 1]] - QLO[q]
                rl[:L] = (dseg[qb[q]:qb[q + 1]] - nb_lo).astype(np.float32)
                rel[c_, :, toff:toff + tqi] = _to_bf16_bits(
                    np.ascontiguousarray(rl.reshape(tqi, P).T))
                relT[c_, 0, P * toff:P * (toff + tqi)] = _to_bf16_bits(rl)
                idxw[c_, :, 8 * toff:8 * (toff + tqi)] = (
                    loc.astype(np.int16).reshape(tqi * 8, 16).T)
                rl_all.append(rl)
                toff += tqi
            # a_dst gather: idx = b*128 + rel (0 for pads), wrapped per chunk
            rl_all = np.concatenate(rl_all) if rl_all else np.zeros(0, np.float32)
            T = int(cfg.tlist[g])
            adix = np.where(rl_all >= 0, b * P + rl_all, 0).astype(np.int16)
            c0 = 0
            while c0 < T:
                tc_ = min(ADCH, T - c0)
                seg = adix[c0 * P:(c0 + tc_) * P]
                idxad[c_, :, 8 * (toff0 + c0):8 * (toff0 + c0 + tc_)] = (
                    seg.reshape(tc_ * 8, 16).T)
                c0 += tc_
            blkbase[c_, g, 0] = nb_lo
            omap.append((nb_lo, n_nodes))
        out_map.append(omap)
    cfg.rel = rel
    cfg.relT = relT
    cfg.idxw = idxw
    cfg.idxad = idxad
    cfg.blkbase = blkbase
    cfg.ADCH = ADCH
    return cfg, None, out_map


def _to_bf16_bits(a_f32):
    import ml_dtypes
    return a_f32.astype(ml_dtypes.bfloat16).view(np.int16)


def build_program(cfg: Cfg, skip_p2=False, reps=1, skip_gather=False, dbg=()):
    dbg = set(dbg)
    nc = bacc.Bacc(num_swdge_queues=NQ)

    xTb = nc.dram_tensor("xTb", [P, N], BF16, kind="ExternalInput")
    wtb = nc.dram_tensor("wtb", [P, P], BF16, kind="ExternalInput")   # W.T bf16
    w_of = nc.dram_tensor("w_of", [P, P], FP32, kind="ExternalInput")  # W fp32
    apat = nc.dram_tensor("apat", [P, 8], FP32, kind="ExternalInput")
    iota_in = nc.dram_tensor("iota_in", [1, P], FP32, kind="ExternalInput")
    iota_col_in = nc.dram_tensor("iota_col", [P, 1], FP32, kind="ExternalInput")
    gamma_c = nc.dram_tensor("gamma_c", [P, 1], FP32, kind="ExternalInput")
    beta_c = nc.dram_tensor("beta_c", [P, 1], FP32, kind="ExternalInput")
    meta_rel = nc.dram_tensor("meta_rel", [P, cfg.TT], I16, kind="ExternalInput")
    metaTb = nc.dram_tensor("metaTb", [1, P * cfg.TT], I16, kind="ExternalInput")
    idx_in = nc.dram_tensor("idx_in", [16, 8 * cfg.TT], I16, kind="ExternalInput")
    blkbase_in = nc.dram_tensor("blkbase", [P, 1], I32, kind="ExternalInput")
    out = nc.dram_tensor("out", [cfg.npos * P, P], FP32, kind="ExternalOutput")

    qsizes = [QLO[q + 1] - QLO[q] for q in range(NQ)]
    g2q = [nc.dram_tensor(f"g2q{q}", [qsizes[q], ROWC], BF16) for q in range(NQ)]
    adstT = nc.dram_tensor("adstT", [cfg.nxt * P, H], FP32)
    ccin = nc.dram_tensor("ccin", [P, 2], FP32)
    ccout = nc.dram_tensor("ccout", [P, 2], FP32)
    scsh = nc.dram_tensor("scsh", [2, P], FP32)

    tmax = cfg.tmax
    GRPL_C = 12

    with tile.TileContext(nc) as tc, ExitStack() as ctx:
        consts = ctx.enter_context(tc.tile_pool(name="consts", bufs=1))
        p1x = ctx.enter_context(tc.tile_pool(name="p1x", bufs=2))
        p1g = ctx.enter_context(tc.tile_pool(name="p1g", bufs=2))
        p1a = ctx.enter_context(tc.tile_pool(name="p1a", bufs=2))
        p1ps = ctx.enter_context(tc.tile_pool(name="p1ps", bufs=1, space="PSUM"))
        mpool = ctx.enter_context(tc.tile_pool(name="mpool", bufs=3))
        ixpool = ctx.enter_context(tc.tile_pool(name="ixpool", bufs=3))
        vpool = ctx.enter_context(tc.tile_pool(name="vpool", bufs=4))
        spool = ctx.enter_context(tc.tile_pool(name="spool", bufs=2))
        adpool = ctx.enter_context(tc.tile_pool(name="adpool", bufs=3))
        stbpool = ctx.enter_context(tc.tile_pool(name="stbpool", bufs=2))
        scpool = ctx.enter_context(tc.tile_pool(name="scpool", bufs=3))
        blkps = ctx.enter_context(tc.tile_pool(name="blkps", bufs=2, space="PSUM"))
        trps = ctx.enter_context(tc.tile_pool(name="trps", bufs=2, space="PSUM"))
        adps = ctx.enter_context(tc.tile_pool(name="adps", bufs=2, space="PSUM"))
        epi = ctx.enter_context(tc.tile_pool(name="epi", bufs=4))
        opre = ctx.enter_context(tc.tile_pool(name="opre", bufs=1))
        ph3 = ctx.enter_context(tc.tile_pool(name="ph3", bufs=3))

        # ---- constants ----
        iota_f = consts.tile([P, P], FP32)
        nc.sync.dma_start(
            out=iota_f[:],
            in_=bass.AP(tensor=iota_in.ap().tensor, offset=0, ap=[[0, P], [1, P]]),
        )
        iota_sb = consts.tile([P, P], BF16)
        nc.vector.tensor_copy(iota_sb[:], iota_f[:])
        piota_col = consts.tile([P, 1], FP32)
        nc.sync.dma_start(out=piota_col[:], in_=iota_col_in[:, :])
        ones_row = consts.tile([1, P], BF16)
        nc.vector.memset(ones_row[:], 1.0)
        ones_col = consts.tile([P, 1], BF16)
        nc.vector.memset(ones_col[:], 1.0)
        gam_sb = consts.tile([P, 1], FP32)
        nc.sync.dma_start(out=gam_sb[:], in_=gamma_c[:, :])
        bet_sb = consts.tile([P, 1], FP32)
        nc.sync.dma_start(out=bet_sb[:], in_=beta_c[:, :])
        rhs_sb = consts.tile([P, 136], BF16)  # [W.T | WA_src | WA_dst] bf16
        nc.sync.dma_start(out=rhs_sb[:, 0:P], in_=wtb[:, :])
        w_sb = consts.tile([P, P], FP32)
        nc.sync.dma_start(out=w_sb[:], in_=w_of[:, :])
        apat_sb = consts.tile([P, 8], FP32)
        nc.sync.dma_start(out=apat_sb[:], in_=apat[:, :])
        wa_ps = p1ps.tile([P, 8], FP32, tag="w8")
        nc.tensor.matmul(out=wa_ps[:], lhsT=w_sb[:], rhs=apat_sb[:], start=True, stop=True)
        nc.scalar.copy(out=rhs_sb[:, P:P + 8], in_=wa_ps[:])

        for _rep in range(reps):
            # ---- phase 1 ----
            GRP = 3
            GRPL = 12
            ntiles = cfg.nxt
            copy_i = 0
            tl = 0
            while tl < ntiles:
                lts = min(GRPL, ntiles - tl)
                nb = tl * P
                ncols = min(lts * P, N - nb)
                xt_sb = p1x.tile([P, GRPL * P], BF16, tag="xt")
                nc.scalar.dma_start(out=xt_sb[:, 0:ncols], in_=xTb[:, nb:nb + ncols])
                if ncols < lts * P:
                    nc.vector.memset(xt_sb[:, ncols:lts * P], 0.0)
                g_sb = p1g.tile([P, GRPL_C, ROWC], BF16, tag="g")
                a_sb = p1a.tile([P, GRPL, H], FP32, tag="a")
                # pad cols (stored to HBM, never consumed) — gpsimd is idle here
                nc.gpsimd.memset(g_sb[:, 0:lts, 132:ROWC], 0.0)
                sl = 0
                while sl < lts:
                    gts = min(GRP, lts - sl)
                    ps = p1ps.tile([P, GRP * 136], FP32, tag="p1")
                    for t in range(gts):
                        nc.tensor.matmul(
                            out=ps[:, t * 136:(t + 1) * 136],
                            lhsT=xt_sb[:, (sl + t) * P:(sl + t) * P + P],
                            rhs=rhs_sb[:],
                            start=True,
                            stop=True,
                        )
                    ps3 = ps[:, 0:gts * 136].rearrange("p (t c) -> p t c", c=136)
                    if copy_i % 2 == 0:
                        nc.scalar.copy(out=g_sb[:, sl:sl + gts, 0:132], in_=ps3[:, :, 0:132])
                        nc.vector.tensor_copy(a_sb[:, sl:sl + gts, :], ps3[:, :, 132:136])
                    else:
                        nc.vector.tensor_copy(g_sb[:, sl:sl + gts, 0:132], ps3[:, :, 0:132])
                        nc.scalar.copy(out=a_sb[:, sl:sl + gts, :], in_=ps3[:, :, 132:136])
                    copy_i += 1
                    sl += gts
                # store: split the load-group at quarter boundaries
                t0 = 0
                while t0 < lts:
                    gtile = tl + t0
                    q = min((gtile * P) // QS, NQ - 1)
                    qend_tile = (QLO[q + 1] + P - 1) // P
                    te = min(lts, qend_tile - tl)
                    rlo = gtile * P - QLO[q]
                    m = min((te - t0) * P, qsizes[q] - rlo, N - gtile * P)
                    nt = (m + P - 1) // P
                    if m % P == 0:
                        nc.sync.dma_start(
                            out=g2q[q][rlo:rlo + m, :].rearrange(
                                "(t p) c -> p t c", t=nt),
                            in_=g_sb[:, t0:t0 + nt, :],
                        )
                    else:
                        for tt in range(nt):
                            mm = min(P, m - tt * P)
                            nc.sync.dma_start(
                                out=g2q[q][rlo + tt * P:rlo + tt * P + mm, :],
                                in_=g_sb[0:mm, t0 + tt, :],
                            )
                    t0 = te
                # a_dst store (full tiles; tail rows >=N are unused padding)
                nc.sync.dma_start(
                    out=adstT[nb:nb + lts * P, :].rearrange(
                        "(t p) c -> p t c", t=lts),
                    in_=a_sb[:, 0:lts, :],
                )
                tl += lts


            # ---- phase 2 ----
            stacc_sb = consts.tile([P, 2], FP32)
            opre_buf = opre.tile([P, cfg.npos * P], BF16)
            if skip_p2:
                nc.vector.memset(stacc_sb[:], 1.0)
                nc.vector.memset(opre_buf[:], 0.5)
            if not skip_p2:
                blkb_sb = consts.tile([P, 1], I32)
                nc.sync.dma_start(out=blkb_sb[:], in_=blkbase_in[:, :])
                bigadb = consts.tile([P, P * H], FP32)
                nc.gpsimd.indirect_dma_start(
                    out=bigadb[:],
                    out_offset=None,
                    in_=adstT[:, :],
                    in_offset=bass.IndirectOffsetOnAxis(ap=blkb_sb[:, 0:1], axis=0),
                )
            for g in range(0 if skip_p2 else cfg.npos):
                T = int(cfg.tlist[g])
                toff = int(cfg.offs[g])

                m_sb = mpool.tile([P, tmax], I16, tag="meta")
                nc.scalar.dma_start(out=m_sb[:, 0:T], in_=meta_rel[:, toff:toff + T])
                relf = m_sb[:, 0:T].bitcast(BF16)

                idx_sb = ixpool.tile([P, tmax * 8], I16, tag="ix")
                nc.sync.dma_start(
                    out=idx_sb[:, 0:T * 8],
                    in_=bass.AP(tensor=idx_in.ap().tensor, offset=8 * toff,
                                ap=[[0, 8], [8 * cfg.TT, 16], [1, 8 * T]]),
                )
                relT_sb = mpool.tile([1, tmax * P], I16, tag="mT")
                nc.scalar.dma_start(
                    out=relT_sb[0:1, 0:T * P],
                    in_=metaTb[0:1, P * toff:P * (toff + T)])
                relT_bf = relT_sb[:, 0:T * P].bitcast(BF16)

                v = vpool.tile([P, tmax, ROWC], BF16, tag="v")
                if skip_gather:
                    nc.vector.memset(v[:, 0:T, 0:132], 0.25)
                else:
                    tq0 = 0
                    for q in range(NQ):
                        tqi = int(cfg.tq[g, q])
                        if tqi == 0:
                            continue
                        nc.gpsimd.dma_gather(
                            out_ap=v[:, tq0:tq0 + tqi, :],
                            in_ap=g2q[q][:, :],
                            idxs_ap=idx_sb[:, 8 * tq0:8 * (tq0 + tqi)],
                            num_idxs=tqi * P,
                            num_idxs_reg=tqi * P,
                            elem_size=ROWC,
                            queue_num=q,
                            single_packet=False,
                        )
                        tq0 += tqi

                # S one-hot [e, x] in bf16
                s_t = spool.tile([P, tmax * P], BF16, tag="s")
                s3 = s_t[:, 0:T * P].rearrange("p (t x) -> p t x", x=P)
                if "nos" not in dbg:
                    nc.vector.tensor_tensor(
                        out=s3,
                        in0=iota_sb[:].unsqueeze(1).broadcast_to((P, T, P)),
                        in1=relf.unsqueeze(2).broadcast_to((P, T, P)),
                        op=mybir.AluOpType.is_equal,
                    )
                else:
                    nc.vector.memset(s_t[:, 0:T * P], 0.0)

                # ST via PE broadcast of rel + is_equal
                st_sb = stbpool.tile([P, tmax * P], BF16, tag="stb")
                CHT = 4
                c0 = 0
                while c0 < T:
                    w_ = min(CHT, T - c0) * P
                    rtp = trps.tile([P, CHT * P], FP32, tag="tr")
                    nc.tensor.matmul(
                        out=rtp[:, 0:w_], lhsT=ones_row[:],
                        rhs=relT_bf[0:1, c0 * P:c0 * P + w_],
                        start=True, stop=True,
                    )
                    nc.vector.tensor_tensor(
                        out=st_sb[:, c0 * P:c0 * P + w_],
                        in0=rtp[:, 0:w_],
                        in1=piota_col[:].broadcast_to((P, w_)),
                        op=mybir.AluOpType.is_equal,
                    )
                    c0 += CHT

                adbf = adpool.tile([P, H], FP32, tag="adbf")
                nc.sync.dma_start(out=adbf[:], in_=bigadb[g:g + 1, 0:P * H])
                adb = adpool.tile([P, H], BF16, tag="adb")
                nc.vector.tensor_copy(adb[:], adbf[:])
                ad_ps = adps.tile([P, tmax * H], FP32, tag="adp")
                for j in range(T):
                    nc.tensor.matmul(
                        out=ad_ps[:, j * H:(j + 1) * H],
                        lhsT=st_sb[:, j * P:(j + 1) * P],
                        rhs=adb[:], start=True, stop=True,
                    )

                # scores -> exp weights (w into v a_src slots, bf16)
                v3 = v[:, 0:T, :]
                sc = scpool.tile([P, tmax * H], FP32, tag="sc")
                sc3 = sc[:, 0:T * H].rearrange("p (t h) -> p t h", h=H)
                nc.vector.tensor_tensor(
                    out=sc3, in0=v3[:, :, P:P + H],
                    in1=ad_ps[:, 0:T * H].rearrange("p (t h) -> p t h", h=H),
                    op=mybir.AluOpType.add,
                )
                sc2 = scpool.tile([P, tmax * H], FP32, tag="sc2")
                nc.vector.scalar_tensor_tensor(
                    out=sc2[:, 0:T * H], in0=sc[:, 0:T * H], scalar=NEG,
                    in1=sc[:, 0:T * H],
                    op0=mybir.AluOpType.mult, op1=mybir.AluOpType.max,
                )
                nc.scalar.activation(
                    out=v3[:, :, P:P + H],
                    in_=sc2[:, 0:T * H].rearrange("p (t h) -> p t h", h=H),
                    func=mybir.ActivationFunctionType.Exp,
                )

                # V' = w * xt (bf16)
                if "novp" not in dbg:
                    v4 = v3[:, :, 0:P].rearrange("p t (h c) -> p t h c", c=C)
                    nc.vector.tensor_tensor(
                        out=v4,
                        in0=v4,
                        in1=v3[:, :, P:P + H].unsqueeze(3).broadcast_to((P, T, H, C)),
                        op=mybir.AluOpType.mult,
                    )

                bps = blkps.tile([P, P + H], FP32, tag="blk")
                if "noagg" not in dbg:
                    for j in range(T):
                        nc.tensor.matmul(
                            out=bps[:],
                            lhsT=s3[:, j, :],
                            rhs=v3[:, j, 0:P + H],
                            start=(j == 0),
                            stop=(j == T - 1),
                        )
                else:
                    nc.tensor.matmul(
                        out=bps[:], lhsT=s3[:, 0, :], rhs=v3[:, 0, 0:P + H],
                        start=True, stop=True,
                    )

                # epilogue
                dmax = epi.tile([P, H], FP32, tag="dmax")
                nc.vector.tensor_scalar_max(dmax[:], bps[:, P:P + H], 1e-30)
                rec = epi.tile([P, H], FP32, tag="rec")
                nc.vector.reciprocal(rec[:], dmax[:])
                op_sl = opre_buf[:, g * P:(g + 1) * P]
                nc.vector.tensor_tensor(
                    out=op_sl.rearrange("p (h c) -> p h c", c=C),
                    in0=bps[:, 0:P].rearrange("p (h c) -> p h c", c=C),
                    in1=rec[:].unsqueeze(2).broadcast_to((P, H, C)),
                    op=mybir.AluOpType.mult,
                )
                sq = epi.tile([P, P], BF16, tag="sq")
                nc.scalar.activation(
                    out=sq[:], in_=op_sl, func=mybir.ActivationFunctionType.Square
                )
                stp = p1ps.tile([P, 8], FP32, tag="w8")
                nc.tensor.matmul(
                    out=stp[:, 0:1], lhsT=op_sl, rhs=ones_col[:], start=True, stop=True,
                )
                nc.tensor.matmul(
                    out=stp[:, 1:2], lhsT=sq[:], rhs=ones_col[:], start=True, stop=True,
                )
                if g == 0:
                    nc.vector.tensor_copy(stacc_sb[:], stp[:, 0:2])
                else:
                    nc.vector.tensor_tensor(
                        out=stacc_sb[:], in0=stacc_sb[:], in1=stp[:, 0:2],
                        op=mybir.AluOpType.add,
                    )

            # ---- phase 3 ----
            nc.sync.dma_start(out=ccin[:, :], in_=stacc_sb[:])
            nc.gpsimd.collective_compute(
                "AllReduce",
                mybir.AluOpType.add,
                replica_groups=[list(range(cfg.ncores))],
                ins=[ccin.ap().opt()],
                outs=[ccout.ap().opt()],
            )
            gst = ph3.tile([P, 2], FP32, tag="gst")
            nc.sync.dma_start(out=gst[:], in_=ccout[:, :])

            ntot = float(cfg.shard * cfg.ncores)
            mean_t = ph3.tile([P, 1], FP32, tag="mean")
            nc.vector.tensor_scalar_mul(mean_t[:], gst[:, 0:1], 1.0 / ntot)
            m2_t = ph3.tile([P, 1], FP32, tag="m2")
            nc.vector.tensor_scalar_mul(m2_t[:], gst[:, 1:2], 1.0 / ntot)
            var_t = ph3.tile([P, 1], FP32, tag="var")
            nc.vector.tensor_tensor(out=var_t[:], in0=mean_t[:], in1=mean_t[:], op=mybir.AluOpType.mult)
            nc.vector.tensor_sub(var_t[:], m2_t[:], var_t[:])
            nc.vector.tensor_scalar_add(var_t[:], var_t[:], EPS)
            sd_t = ph3.tile([P, 1], FP32, tag="sd")
            nc.scalar.activation(out=sd_t[:], in_=var_t[:], func=mybir.ActivationFunctionType.Sqrt)
            rinv_t = ph3.tile([P, 1], FP32, tag="rinv")
            nc.vector.reciprocal(rinv_t[:], sd_t[:])
            sc_t = ph3.tile([P, 1], FP32, tag="sct")
            nc.vector.tensor_tensor(out=sc_t[:], in0=rinv_t[:], in1=gam_sb[:], op=mybir.AluOpType.mult)
            sh_t = ph3.tile([P, 1], FP32, tag="sht")
            nc.vector.tensor_tensor(out=sh_t[:], in0=mean_t[:], in1=sc_t[:], op=mybir.AluOpType.mult)
            nc.vector.tensor_sub(sh_t[:], bet_sb[:], sh_t[:])

            nc.sync.dma_start(out=scsh[0:1, :], in_=sc_t[:])
            nc.sync.dma_start(out=scsh[1:2, :], in_=sh_t[:])
            screp = consts.tile([P, P], FP32)
            nc.sync.dma_start(
                out=screp[:],
                in_=bass.AP(tensor=scsh.ap().tensor, offset=0, ap=[[0, P], [1, P]]),
            )
            shrep = consts.tile([P, P], FP32)
            nc.sync.dma_start(
                out=shrep[:],
                in_=bass.AP(tensor=scsh.ap().tensor, offset=P, ap=[[0, P], [1, P]]),
            )

            CH = 4
            g = 0
            while g < cfg.npos:
                k = min(CH, cfg.npos - g)
                op_sl = opre_buf[:, g * P:(g + k) * P].rearrange(
                    "p (k c) -> p k c", c=P)
                t0 = ph3.tile([P, CH * P], FP32, tag="t0")
                t03 = t0[:, 0:k * P].rearrange("p (k c) -> p k c", c=P)
                nc.vector.tensor_tensor(
                    out=t03, in0=op_sl,
                    in1=screp[:].unsqueeze(1).broadcast_to((P, k, P)),
                    op=mybir.AluOpType.mult)
                nc.vector.tensor_tensor(
                    out=t03, in0=t03,
                    in1=shrep[:].unsqueeze(1).broadcast_to((P, k, P)),
                    op=mybir.AluOpType.add)
                t1 = ph3.tile([P, CH * P], FP32, tag="t1")
                nc.vector.tensor_scalar_mul(t1[:, 0:k * P], t0[:, 0:k * P], NEG)
                nc.vector.tensor_tensor(
                    out=t1[:, 0:k * P], in0=t1[:, 0:k * P],
                    in1=t0[:, 0:k * P], op=mybir.AluOpType.max)
                nc.sync.dma_start(
                    out=out[g * P:(g + k) * P, :].rearrange("(k p) c -> p k c", k=k),
                    in_=t1[:, 0:k * P].rearrange("p (k c) -> p k c", c=P),
                )
                g += k

    nc.compile()
    return nc


def make_inputs(x, W, att_src, att_dst, gamma, beta, meta, cfg: Cfg):
    import ml_dtypes
    x = np.asarray(x, np.float32)
    W = np.asarray(W, np.float32)
    att_src = np.asarray(att_src, np.float32)
    att_dst = np.asarray(att_dst, np.float32)
    apat = np.zeros((P, 8), np.float32)
    for h in range(H):
        apat[h * C:(h + 1) * C, h] = att_src[h]
        apat[h * C:(h + 1) * C, 4 + h] = att_dst[h]
    xTb = np.ascontiguousarray(x.T).astype(ml_dtypes.bfloat16)
    wtb = np.ascontiguousarray(W.T).astype(ml_dtypes.bfloat16)
    iota = np.arange(P, dtype=np.float32).reshape(1, P)
    gam = np.asarray(gamma, np.float32).reshape(P, 1)
    bet = np.asarray(beta, np.float32).reshape(P, 1)
    in_maps = []
    for c_ in range(cfg.ncores):
        in_maps.append(
            {
                "xTb": xTb,
                "wtb": wtb,
                "w_of": W,
                "apat": apat,
                "iota_in": iota,
                "iota_col": np.arange(P, dtype=np.float32).reshape(P, 1),
                "gamma_c": gam,
                "beta_c": bet,
                "meta_rel": np.ascontiguousarray(cfg.rel[c_]),
                "metaTb": np.ascontiguousarray(cfg.relT[c_]),
                "idx_in": np.ascontiguousarray(cfg.idxw[c_]),
                "blkbase": np.ascontiguousarray(cfg.blkbase[c_]),
            }
        )
    return in_maps


def assemble_output(core_outs, out_map, cfg: Cfg, n):
    full = np.empty((n, P), np.float32)
    for c_ in range(cfg.ncores):
        for g, (nb_lo, n_valid) in enumerate(out_map[c_]):
            if n_valid > 0:
                full[nb_lo:nb_lo + n_valid] = core_outs[c_][g * P:g * P + n_valid]
    return full


def kernel(**inputs) -> np.ndarray:
    from concourse.bass_utils import run_bass_kernel_spmd

    cfg, _, out_map = preprocess(inputs["edge_index"])
    nc = build_program(cfg)
    in_maps = make_inputs(
        inputs["x"], inputs["W"], inputs["att_src"], inputs["att_dst"],
        inputs["gamma"], inputs["beta"], None, cfg,
    )
    res = run_bass_kernel_spmd(nc, in_maps, core_ids=list(range(NCORES)))
    core_outs = [res.results[c_]["out"] for c_ in range(NCORES)]
    return assemble_output(core_outs, out_map, cfg, N)


# revision 45
# speedup vs baseline: 1.4027x; 1.4027x over previous
"""GATConv block (GAT attention + BatchNorm + leaky_relu) on 8 Trainium2 NeuronCores.

v2 strategy (graph/data parallel):
- Nodes sharded across 8 cores by destination range (12500 each).
- Phase 1 (replicated): each core computes the full feature table
  row[n] = [xt[n] (128) | a_src[n] (4) | pad (120)] in bf16, written as
  512B rows into 4 quarter-tables (so gather indices fit int16), plus a
  separate fp32 a_dst table.
- Phase 2: edges grouped by dst block (<=128 dst nodes/position), sorted by
  src within the block, split into 4 runs by src quarter.  Each run is ONE
  dma_gather (512B rows) on its own SWDGE queue (4 queues drain in parallel
  -- ~5x the single-queue indirect-DMA rate).  Per-edge a_dst comes from a
  one-hot lookup built on PE+DVE from host-provided dst offsets; attention
  weights exp(leaky(a_src+a_dst)) multiply xt in bf16 on DVE, and a per-tile
  PE matmul  out[dst_block] += S.T @ [w*xt | w]  accumulates numerator and
  softmax denominator in fp32 PSUM.
- Phase 3: BatchNorm batch stats via ones-vector matmuls, AllReduce across
  cores, per-channel affine + leaky_relu.

All data-dependent structure is host data; tile counts per (position, quarter)
are equalized across cores (position-wise sort + max padding), with pad gather
slots pointing at row 0 (gated out by rel=-1 one-hot misses).
"""

import sys

sys.path.insert(0, "/opt/trn_rl_repo")

import numpy as np
from contextlib import ExitStack

import concourse.bass as bass
import concourse.mybir as mybir
import concourse.tile as tile
from concourse import bacc

# --- queue-aware SWDGE semaphore-lane assignment -------------------------
# Tile round-robins Pool-engine DMA completion sems over 8 DMASW lanes with
# no regard for the SWDGE queue, but a sem may only be incremented from one
# queue.  Dedicate lanes {2q, 2q+1} to queue q (InstDMACopy et al. without a
# queue_num always execute on queue 0).
import concourse.tile_sem_assignment as _tsa
import concourse.bass_isa as _bass_isa

if not getattr(_tsa, "_qaware_patch", False):
    _tsa._qaware_patch = True
    _DMAInst = _tsa.DMAInst

    _orig_assign_tick = _tsa.TileClockTick._assign_tick

    def _assign_tick_qaware(self, inst):
        if (
            isinstance(inst, _DMAInst)
            and not isinstance(inst, _bass_isa.UserSyncedRemoteDMADescs)
            and inst.engine == mybir.EngineType.Pool
        ):
            qn = getattr(inst, "queue_num", 0) or 0
            rot = getattr(self, "_qaware_rot", None)
            if rot is None:
                rot = self._qaware_rot = {}
            r = rot.get(qn, 0)
            rot[qn] = r ^ 1
            self.next_sw_dma_idx = (2 * qn + r) % self.swdge_sem_count
        return _orig_assign_tick(self, inst)

    _tsa.TileClockTick._assign_tick = _assign_tick_qaware
# -------------------------------------------------------------------------

FP32 = mybir.dt.float32
BF16 = mybir.dt.bfloat16
I32 = mybir.dt.int32
I16 = mybir.dt.int16

N = 100000
E = 1600000
F_IN = 128
H = 4
C = 32
F_OUT = H * C
NEG = 0.2
EPS = 1e-5
NCORES = 8
P = 128
ROWC = 256          # table row: [xt 128 | a_src 4 | pad] bf16 = 512B
QS = 25088          # quarter size (multiple of 128, < 32768 for int16 idx)
QLO = [0, QS, 2 * QS, 3 * QS, N]
NQ = 4


class Cfg:
    def __init__(self, npos, tq, shard, ncores):
        self.npos = npos            # positions per core
        self.tq = tq                # [npos, NQ] tiles per (position, quarter)
        self.tlist = tq.sum(axis=1)  # [npos] total tiles per position
        self.shard = shard
        self.ncores = ncores
        self.TT = int(self.tlist.sum())
        self.offs = np.concatenate([[0], np.cumsum(self.tlist)]).astype(int)
        self.tmax = int(self.tlist.max())
        self.nxt = (N + P - 1) // P


def preprocess(edge_index, n=N, ncores=NCORES):
    src = np.asarray(edge_index[0]).astype(np.int64)
    dst = np.asarray(edge_index[1]).astype(np.int64)
    order = np.argsort(dst, kind="stable")
    s_src = src[order].astype(np.int32)
    s_dst = dst[order].astype(np.int32)

    shard = n // ncores
    npos = (shard + P - 1) // P

    # per core, per block: edge range + per-quarter run lengths
    blocks = []   # [c][b] = (nb_lo, n_nodes, e_lo, e_hi, qlens[4])
    for c_ in range(ncores):
        lo_n = c_ * shard
        hi_n = lo_n + shard
        bl = []
        for b in range(npos):
            nb_lo = lo_n + b * P
            nb_hi = min(nb_lo + P, hi_n)
            e_lo = int(np.searchsorted(s_dst, nb_lo, "left"))
            e_hi = int(np.searchsorted(s_dst, nb_hi, "left"))
            sseg = s_src[e_lo:e_hi]
            o2 = np.argsort(sseg, kind="stable")
            ss = sseg[o2]
            qb = [int(np.searchsorted(ss, QLO[q], "left")) for q in range(5)]
            qlens = [qb[q + 1] - qb[q] for q in range(4)]
            bl.append((nb_lo, nb_hi - nb_lo, e_lo, e_hi, o2, qlens))
        blocks.append(bl)

    tiles = np.zeros((ncores, npos), np.int64)
    tq_all = np.zeros((ncores, npos, NQ), np.int64)
    for c_ in range(ncores):
        for b in range(npos):
            ql = blocks[c_][b][5]
            for q in range(NQ):
                tq_all[c_, b, q] = -(-ql[q] // P)
            tiles[c_, b] = tq_all[c_, b].sum()

    # balance: per core sort blocks by total tiles desc, then per-(slot, q) max
    perm = np.argsort(-tiles, axis=1, kind="stable")
    tq_sorted = np.take_along_axis(
        tq_all, perm[:, :, None], axis=1)      # [c, g, q]
    tq_max = tq_sorted.max(axis=0)             # [g, q]
    # every position needs >=1 tile (pads are gated by rel=-1)
    empty = tq_max.sum(axis=1) == 0
    tq_max[empty, 0] = 1

    cfg = Cfg(npos, tq_max, shard, ncores)

    # host meta, per core:
    #  rel   [128, TT] int32 (fp32 bits; -1.0 for pads)
    #  idxw  [16, 8*TT] int16 (xt gather indices, 16-wrapped per run)
    #  idxad [16, 8*TT] int16 (a_dst gather indices = block*128+rel, wrapped
    #         per ADCH-tile chunk; 0 for pads)
    ADCH = 32
    rel = np.zeros((ncores, P, cfg.TT), np.int32)
    relT = np.zeros((ncores, 1, P * cfg.TT), np.int16)
    idxw = np.zeros((ncores, 16, 8 * cfg.TT), np.int16)
    idxad = np.zeros((ncores, 16, 8 * cfg.TT), np.int16)
    blkbase = np.zeros((ncores, P, 1), np.int32)
    out_map = []
    for c_ in range(ncores):
        omap = []
        for g in range(npos):
            b = int(perm[c_, g])
            nb_lo, n_nodes, e_lo, e_hi, o2, qlens = blocks[c_][b]
            sseg = s_src[e_lo:e_hi][o2]
            dseg = s_dst[e_lo:e_hi][o2]
            qb = np.concatenate([[0], np.cumsum(qlens)]).astype(int)
            toff0 = int(cfg.offs[g])
            toff = toff0
            rl_all = []
            for q in range(NQ):
                tqi = int(tq_max[g, q])
                if tqi == 0:
                    continue
                L = qlens[q]
                npad = tqi * P
                loc = np.zeros(npad, np.int32)
                rl = np.full(npad, -1.0, np.float32)
                loc[:L] = sseg[qb[q]:qb[q + 1]] - QLO[q]
                rl[:L] = (dseg[qb[q]:qb[q + 1]] - nb_lo).astype(np.float32)
                rel[c_, :, toff:toff + tqi] = rl.reshape(tqi, P).T.view(np.int32)
                relT[c_, 0, P * toff:P * (toff + tqi)] = _to_bf16_bits(rl)
                idxw[c_, :, 8 * toff:8 * (toff + tqi)] = (
                    loc.astype(np.int16).reshape(tqi * 8, 16).T)
                rl_all.append(rl)
                toff += tqi
            # a_dst gather: idx = b*128 + rel (0 for pads), wrapped per chunk
            rl_all = np.concatenate(rl_all) if rl_all else np.zeros(0, np.float32)
            T = int(cfg.tlist[g])
            adix = np.where(rl_all >= 0, b * P + rl_all, 0).astype(np.int16)
            c0 = 0
            while c0 < T:
                tc_ = min(ADCH, T - c0)
                seg = adix[c0 * P:(c0 + tc_) * P]
                idxad[c_, :, 8 * (toff0 + c0):8 * (toff0 + c0 + tc_)] = (
                    seg.reshape(tc_ * 8, 16).T)
                c0 += tc_
            blkbase[c_, g, 0] = nb_lo
            omap.append((nb_lo, n_nodes))
        out_map.append(omap)
    cfg.rel = rel
    cfg.relT = relT
    cfg.idxw = idxw
    cfg.idxad = idxad
    cfg.blkbase = blkbase
    cfg.ADCH = ADCH
    return cfg, None, out_map


def _to_bf16_bits(a_f32):
    import ml_dtypes
    return a_f32.astype(ml_dtypes.bfloat16).view(np.int16)


def build_program(cfg: Cfg, skip_p2=False, reps=1, skip_gather=False, dbg=()):
    dbg = set(dbg)
    nc = bacc.Bacc(num_swdge_queues=NQ)

    xTb = nc.dram_tensor("xTb", [P, N], BF16, kind="ExternalInput")
    wtb = nc.dram_tensor("wtb", [P, P], BF16, kind="ExternalInput")   # W.T bf16
    w_of = nc.dram_tensor("w_of", [P, P], FP32, kind="ExternalInput")  # W fp32
    apat = nc.dram_tensor("apat", [P, 8], FP32, kind="ExternalInput")
    iota_in = nc.dram_tensor("iota_in", [1, P], FP32, kind="ExternalInput")
    iota_col_in = nc.dram_tensor("iota_col", [P, 1], FP32, kind="ExternalInput")
    gamma_c = nc.dram_tensor("gamma_c", [P, 1], FP32, kind="ExternalInput")
    beta_c = nc.dram_tensor("beta_c", [P, 1], FP32, kind="ExternalInput")
    meta_rel = nc.dram_tensor("meta_rel", [P, cfg.TT], I32, kind="ExternalInput")
    metaTb = nc.dram_tensor("metaTb", [1, P * cfg.TT], I16, kind="ExternalInput")
    idx_in = nc.dram_tensor("idx_in", [16, 8 * cfg.TT], I16, kind="ExternalInput")
    blkbase_in = nc.dram_tensor("blkbase", [P, 1], I32, kind="ExternalInput")
    out = nc.dram_tensor("out", [cfg.npos * P, P], FP32, kind="ExternalOutput")

    qsizes = [QLO[q + 1] - QLO[q] for q in range(NQ)]
    g2q = [nc.dram_tensor(f"g2q{q}", [qsizes[q], ROWC], BF16) for q in range(NQ)]
    adstT = nc.dram_tensor("adstT", [cfg.nxt * P, H], FP32)
    ccin = nc.dram_tensor("ccin", [P, 2], FP32)
    ccout = nc.dram_tensor("ccout", [P, 2], FP32)
    scsh = nc.dram_tensor("scsh", [2, P], FP32)

    tmax = cfg.tmax
    GRPL_C = 12

    with tile.TileContext(nc) as tc, ExitStack() as ctx:
        consts = ctx.enter_context(tc.tile_pool(name="consts", bufs=1))
        p1x = ctx.enter_context(tc.tile_pool(name="p1x", bufs=2))
        p1g = ctx.enter_context(tc.tile_pool(name="p1g", bufs=2))
        p1a = ctx.enter_context(tc.tile_pool(name="p1a", bufs=2))
        p1ps = ctx.enter_context(tc.tile_pool(name="p1ps", bufs=1, space="PSUM"))
        mpool = ctx.enter_context(tc.tile_pool(name="mpool", bufs=3))
        ixpool = ctx.enter_context(tc.tile_pool(name="ixpool", bufs=3))
        vpool = ctx.enter_context(tc.tile_pool(name="vpool", bufs=3))
        spool = ctx.enter_context(tc.tile_pool(name="spool", bufs=2))
        adpool = ctx.enter_context(tc.tile_pool(name="adpool", bufs=3))
        stbpool = ctx.enter_context(tc.tile_pool(name="stbpool", bufs=2))
        scpool = ctx.enter_context(tc.tile_pool(name="scpool", bufs=3))
        blkps = ctx.enter_context(tc.tile_pool(name="blkps", bufs=2, space="PSUM"))
        trps = ctx.enter_context(tc.tile_pool(name="trps", bufs=2, space="PSUM"))
        adps = ctx.enter_context(tc.tile_pool(name="adps", bufs=2, space="PSUM"))
        epi = ctx.enter_context(tc.tile_pool(name="epi", bufs=4))
        opre = ctx.enter_context(tc.tile_pool(name="opre", bufs=1))
        ph3 = ctx.enter_context(tc.tile_pool(name="ph3", bufs=3))

        # ---- constants ----
        iota_sb = consts.tile([P, P], FP32)
        nc.sync.dma_start(
            out=iota_sb[:],
            in_=bass.AP(tensor=iota_in.ap().tensor, offset=0, ap=[[0, P], [1, P]]),
        )
        piota_col = consts.tile([P, 1], FP32)
        nc.sync.dma_start(out=piota_col[:], in_=iota_col_in[:, :])
        ones_row = consts.tile([1, P], BF16)
        nc.vector.memset(ones_row[:], 1.0)
        ones_col = consts.tile([P, 1], BF16)
        nc.vector.memset(ones_col[:], 1.0)
        gam_sb = consts.tile([P, 1], FP32)
        nc.sync.dma_start(out=gam_sb[:], in_=gamma_c[:, :])
        bet_sb = consts.tile([P, 1], FP32)
        nc.sync.dma_start(out=bet_sb[:], in_=beta_c[:, :])
        rhs_sb = consts.tile([P, 136], BF16)  # [W.T | WA_src | WA_dst] bf16
        nc.sync.dma_start(out=rhs_sb[:, 0:P], in_=wtb[:, :])
        w_sb = consts.tile([P, P], FP32)
        nc.sync.dma_start(out=w_sb[:], in_=w_of[:, :])
        apat_sb = consts.tile([P, 8], FP32)
        nc.sync.dma_start(out=apat_sb[:], in_=apat[:, :])
        wa_ps = p1ps.tile([P, 8], FP32, tag="w8")
        nc.tensor.matmul(out=wa_ps[:], lhsT=w_sb[:], rhs=apat_sb[:], start=True, stop=True)
        nc.scalar.copy(out=rhs_sb[:, P:P + 8], in_=wa_ps[:])

        for _rep in range(reps):
            # ---- phase 1 ----
            GRP = 3
            GRPL = 12
            ntiles = cfg.nxt
            copy_i = 0
            tl = 0
            while tl < ntiles:
                lts = min(GRPL, ntiles - tl)
                nb = tl * P
                ncols = min(lts * P, N - nb)
                xt_sb = p1x.tile([P, GRPL * P], BF16, tag="xt")
                nc.scalar.dma_start(out=xt_sb[:, 0:ncols], in_=xTb[:, nb:nb + ncols])
                if ncols < lts * P:
                    nc.vector.memset(xt_sb[:, ncols:lts * P], 0.0)
                g_sb = p1g.tile([P, GRPL_C, ROWC], BF16, tag="g")
                a_sb = p1a.tile([P, GRPL, H], FP32, tag="a")
                # pad cols (stored to HBM, never consumed) — gpsimd is idle here
                nc.gpsimd.memset(g_sb[:, 0:lts, 132:ROWC], 0.0)
                sl = 0
                while sl < lts:
                    gts = min(GRP, lts - sl)
                    ps = p1ps.tile([P, GRP * 136], FP32, tag="p1")
                    for t in range(gts):
                        nc.tensor.matmul(
                            out=ps[:, t * 136:(t + 1) * 136],
                            lhsT=xt_sb[:, (sl + t) * P:(sl + t) * P + P],
                            rhs=rhs_sb[:],
                            start=True,
                            stop=True,
                        )
                    ps3 = ps[:, 0:gts * 136].rearrange("p (t c) -> p t c", c=136)
                    if copy_i % 2 == 0:
                        nc.scalar.copy(out=g_sb[:, sl:sl + gts, 0:132], in_=ps3[:, :, 0:132])
                        nc.vector.tensor_copy(a_sb[:, sl:sl + gts, :], ps3[:, :, 132:136])
                    else:
                        nc.vector.tensor_copy(g_sb[:, sl:sl + gts, 0:132], ps3[:, :, 0:132])
                        nc.scalar.copy(out=a_sb[:, sl:sl + gts, :], in_=ps3[:, :, 132:136])
                    copy_i += 1
                    sl += gts
                # store: split the load-group at quarter boundaries
                t0 = 0
                while t0 < lts:
                    gtile = tl + t0
                    q = min((gtile * P) // QS, NQ - 1)
                    qend_tile = (QLO[q + 1] + P - 1) // P
                    te = min(lts, qend_tile - tl)
                    rlo = gtile * P - QLO[q]
                    m = min((te - t0) * P, qsizes[q] - rlo, N - gtile * P)
                    nt = (m + P - 1) // P
                    if m % P == 0:
                        nc.sync.dma_start(
                            out=g2q[q][rlo:rlo + m, :].rearrange(
                                "(t p) c -> p t c", t=nt),
                            in_=g_sb[:, t0:t0 + nt, :],
                        )
                    else:
                        for tt in range(nt):
                            mm = min(P, m - tt * P)
                            nc.sync.dma_start(
                                out=g2q[q][rlo + tt * P:rlo + tt * P + mm, :],
                                in_=g_sb[0:mm, t0 + tt, :],
                            )
                    t0 = te
                # a_dst store (full tiles; tail rows >=N are unused padding)
                nc.sync.dma_start(
                    out=adstT[nb:nb + lts * P, :].rearrange(
                        "(t p) c -> p t c", t=lts),
                    in_=a_sb[:, 0:lts, :],
                )
                tl += lts


            # ---- phase 2 ----
            stacc_sb = consts.tile([P, 2], FP32)
            opre_buf = opre.tile([P, cfg.npos * P], BF16)
            if skip_p2:
                nc.vector.memset(stacc_sb[:], 1.0)
                nc.vector.memset(opre_buf[:], 0.5)
            if not skip_p2:
                blkb_sb = consts.tile([P, 1], I32)
                nc.sync.dma_start(out=blkb_sb[:], in_=blkbase_in[:, :])
                bigadb = consts.tile([P, P * H], FP32)
                nc.gpsimd.indirect_dma_start(
                    out=bigadb[:],
                    out_offset=None,
                    in_=adstT[:, :],
                    in_offset=bass.IndirectOffsetOnAxis(ap=blkb_sb[:, 0:1], axis=0),
                )
            for g in range(0 if skip_p2 else cfg.npos):
                T = int(cfg.tlist[g])
                toff = int(cfg.offs[g])

                m_sb = mpool.tile([P, tmax], I32, tag="meta")
                nc.scalar.dma_start(out=m_sb[:, 0:T], in_=meta_rel[:, toff:toff + T])
                relf = m_sb[:, 0:T].bitcast(FP32)

                idx_sb = ixpool.tile([P, tmax * 8], I16, tag="ix")
                nc.sync.dma_start(
                    out=idx_sb[:, 0:T * 8],
                    in_=bass.AP(tensor=idx_in.ap().tensor, offset=8 * toff,
                                ap=[[0, 8], [8 * cfg.TT, 16], [1, 8 * T]]),
                )
                relT_sb = mpool.tile([1, tmax * P], I16, tag="mT")
                nc.scalar.dma_start(
                    out=relT_sb[0:1, 0:T * P],
                    in_=metaTb[0:1, P * toff:P * (toff + T)])
                relT_bf = relT_sb[:, 0:T * P].bitcast(BF16)

                v = vpool.tile([P, tmax, ROWC], BF16, tag="v")
                if skip_gather:
                    nc.vector.memset(v[:, 0:T, 0:132], 0.25)
                else:
                    tq0 = 0
                    for q in range(NQ):
                        tqi = int(cfg.tq[g, q])
                        if tqi == 0:
                            continue
                        nc.gpsimd.dma_gather(
                            out_ap=v[:, tq0:tq0 + tqi, :],
                            in_ap=g2q[q][:, :],
                            idxs_ap=idx_sb[:, 8 * tq0:8 * (tq0 + tqi)],
                            num_idxs=tqi * P,
                            num_idxs_reg=tqi * P,
                            elem_size=ROWC,
                            queue_num=q,
                            single_packet=False,
                        )
                        tq0 += tqi

                # S one-hot [e, x] in bf16
                s_t = spool.tile([P, tmax * P], BF16, tag="s")
                s3 = s_t[:, 0:T * P].rearrange("p (t x) -> p t x", x=P)
                if "nos" not in dbg:
                    nc.vector.tensor_tensor(
                        out=s3,
                        in0=iota_sb[:].unsqueeze(1).broadcast_to((P, T, P)),
                        in1=relf.unsqueeze(2).broadcast_to((P, T, P)),
                        op=mybir.AluOpType.is_equal,
                    )
                else:
                    nc.vector.memset(s_t[:, 0:T * P], 0.0)

                # ST via PE broadcast of rel + is_equal
                st_sb = stbpool.tile([P, tmax * P], BF16, tag="stb")
                CHT = 4
                c0 = 0
                while c0 < T:
                    w_ = min(CHT, T - c0) * P
                    rtp = trps.tile([P, CHT * P], FP32, tag="tr")
                    nc.tensor.matmul(
                        out=rtp[:, 0:w_], lhsT=ones_row[:],
                        rhs=relT_bf[0:1, c0 * P:c0 * P + w_],
                        start=True, stop=True,
                    )
                    nc.vector.tensor_tensor(
                        out=st_sb[:, c0 * P:c0 * P + w_],
                        in0=rtp[:, 0:w_],
                        in1=piota_col[:].broadcast_to((P, w_)),
                        op=mybir.AluOpType.is_equal,
                    )
                    c0 += CHT

                adbf = adpool.tile([P, H], FP32, tag="adbf")
                nc.sync.dma_start(out=adbf[:], in_=bigadb[g:g + 1, 0:P * H])
                adb = adpool.tile([P, H], BF16, tag="adb")
                nc.vector.tensor_copy(adb[:], adbf[:])
                ad_ps = adps.tile([P, tmax * H], FP32, tag="adp")
                for j in range(T):
                    nc.tensor.matmul(
                        out=ad_ps[:, j * H:(j + 1) * H],
                        lhsT=st_sb[:, j * P:(j + 1) * P],
                        rhs=adb[:], start=True, stop=True,
                    )

                # scores -> exp weights (w into v a_src slots, bf16)
                v3 = v[:, 0:T, :]
                sc = scpool.tile([P, tmax * H], FP32, tag="sc")
                sc3 = sc[:, 0:T * H].rearrange("p (t h) -> p t h", h=H)
                nc.vector.tensor_tensor(
                    out=sc3, in0=v3[:, :, P:P + H],
                    in1=ad_ps[:, 0:T * H].rearrange("p (t h) -> p t h", h=H),
                    op=mybir.AluOpType.add,
                )
                sc2 = scpool.tile([P, tmax * H], FP32, tag="sc2")
                nc.vector.scalar_tensor_tensor(
                    out=sc2[:, 0:T * H], in0=sc[:, 0:T * H], scalar=NEG,
                    in1=sc[:, 0:T * H],
                    op0=mybir.AluOpType.mult, op1=mybir.AluOpType.max,
                )
                nc.scalar.activation(
                    out=v3[:, :, P:P + H],
                    in_=sc2[:, 0:T * H].rearrange("p (t h) -> p t h", h=H),
                    func=mybir.ActivationFunctionType.Exp,
                )

                # V' = w * xt (bf16)
                if "novp" not in dbg:
                    v4 = v3[:, :, 0:P].rearrange("p t (h c) -> p t h c", c=C)
                    nc.vector.tensor_tensor(
                        out=v4,
                        in0=v4,
                        in1=v3[:, :, P:P + H].unsqueeze(3).broadcast_to((P, T, H, C)),
                        op=mybir.AluOpType.mult,
                    )

                bps = blkps.tile([P, P + H], FP32, tag="blk")
                if "noagg" not in dbg:
                    for j in range(T):
                        nc.tensor.matmul(
                            out=bps[:],
                            lhsT=s3[:, j, :],
                            rhs=v3[:, j, 0:P + H],
                            start=(j == 0),
                            stop=(j == T - 1),
                        )
                else:
                    nc.tensor.matmul(
                        out=bps[:], lhsT=s3[:, 0, :], rhs=v3[:, 0, 0:P + H],
                        start=True, stop=True,
                    )

                # epilogue
                dmax = epi.tile([P, H], FP32, tag="dmax")
                nc.vector.tensor_scalar_max(dmax[:], bps[:, P:P + H], 1e-30)
                rec = epi.tile([P, H], FP32, tag="rec")
                nc.vector.reciprocal(rec[:], dmax[:])
                op_sl = opre_buf[:, g * P:(g + 1) * P]
                nc.vector.tensor_tensor(
                    out=op_sl.rearrange("p (h c) -> p h c", c=C),
                    in0=bps[:, 0:P].rearrange("p (h c) -> p h c", c=C),
                    in1=rec[:].unsqueeze(2).broadcast_to((P, H, C)),
                    op=mybir.AluOpType.mult,
                )
                sq = epi.tile([P, P], BF16, tag="sq")
                nc.scalar.activation(
                    out=sq[:], in_=op_sl, func=mybir.ActivationFunctionType.Square
                )
                stp = p1ps.tile([P, 8], FP32, tag="w8")
                nc.tensor.matmul(
                    out=stp[:, 0:1], lhsT=op_sl, rhs=ones_col[:], start=True, stop=True,
                )
                nc.tensor.matmul(
                    out=stp[:, 1:2], lhsT=sq[:], rhs=ones_col[:], start=True, stop=True,
                )
                if g == 0:
                    nc.vector.tensor_copy(stacc_sb[:], stp[:, 0:2])
                else:
                    nc.vector.tensor_tensor(
                        out=stacc_sb[:], in0=stacc_sb[:], in1=stp[:, 0:2],
                        op=mybir.AluOpType.add,
                    )

            # ---- phase 3 ----
            nc.sync.dma_start(out=ccin[:, :], in_=stacc_sb[:])
            nc.gpsimd.collective_compute(
                "AllReduce",
                mybir.AluOpType.add,
                replica_groups=[list(range(cfg.ncores))],
                ins=[ccin.ap().opt()],
                outs=[ccout.ap().opt()],
            )
            gst = ph3.tile([P, 2], FP32, tag="gst")
            nc.sync.dma_start(out=gst[:], in_=ccout[:, :])

            ntot = float(cfg.shard * cfg.ncores)
            mean_t = ph3.tile([P, 1], FP32, tag="mean")
            nc.vector.tensor_scalar_mul(mean_t[:], gst[:, 0:1], 1.0 / ntot)
            m2_t = ph3.tile([P, 1], FP32, tag="m2")
            nc.vector.tensor_scalar_mul(m2_t[:], gst[:, 1:2], 1.0 / ntot)
            var_t = ph3.tile([P, 1], FP32, tag="var")
            nc.vector.tensor_tensor(out=var_t[:], in0=mean_t[:], in1=mean_t[:], op=mybir.AluOpType.mult)
            nc.vector.tensor_sub(var_t[:], m2_t[:], var_t[:])
            nc.vector.tensor_scalar_add(var_t[:], var_t[:], EPS)
            sd_t = ph3.tile([P, 1], FP32, tag="sd")
            nc.scalar.activation(out=sd_t[:], in_=var_t[:], func=mybir.ActivationFunctionType.Sqrt)
            rinv_t = ph3.tile([P, 1], FP32, tag="rinv")
            nc.vector.reciprocal(rinv_t[:], sd_t[:])
            sc_t = ph3.tile([P, 1], FP32, tag="sct")
            nc.vector.tensor_tensor(out=sc_t[:], in0=rinv_t[:], in1=gam_sb[:], op=mybir.AluOpType.mult)
            sh_t = ph3.tile([P, 1], FP32, tag="sht")
            nc.vector.tensor_tensor(out=sh_t[:], in0=mean_t[:], in1=sc_t[:], op=mybir.AluOpType.mult)
            nc.vector.tensor_sub(sh_t[:], bet_sb[:], sh_t[:])

            nc.sync.dma_start(out=scsh[0:1, :], in_=sc_t[:])
            nc.sync.dma_start(out=scsh[1:2, :], in_=sh_t[:])
            screp = consts.tile([P, P], FP32)
            nc.sync.dma_start(
                out=screp[:],
                in_=bass.AP(tensor=scsh.ap().tensor, offset=0, ap=[[0, P], [1, P]]),
            )
            shrep = consts.tile([P, P], FP32)
            nc.sync.dma_start(
                out=shrep[:],
                in_=bass.AP(tensor=scsh.ap().tensor, offset=P, ap=[[0, P], [1, P]]),
            )

            CH = 4
            g = 0
            while g < cfg.npos:
                k = min(CH, cfg.npos - g)
                op_sl = opre_buf[:, g * P:(g + k) * P].rearrange(
                    "p (k c) -> p k c", c=P)
                t0 = ph3.tile([P, CH * P], FP32, tag="t0")
                t03 = t0[:, 0:k * P].rearrange("p (k c) -> p k c", c=P)
                nc.vector.tensor_tensor(
                    out=t03, in0=op_sl,
                    in1=screp[:].unsqueeze(1).broadcast_to((P, k, P)),
                    op=mybir.AluOpType.mult)
                nc.vector.tensor_tensor(
                    out=t03, in0=t03,
                    in1=shrep[:].unsqueeze(1).broadcast_to((P, k, P)),
                    op=mybir.AluOpType.add)
                t1 = ph3.tile([P, CH * P], FP32, tag="t1")
                nc.vector.tensor_scalar_mul(t1[:, 0:k * P], t0[:, 0:k * P], NEG)
                nc.vector.tensor_tensor(
                    out=t1[:, 0:k * P], in0=t1[:, 0:k * P],
                    in1=t0[:, 0:k * P], op=mybir.AluOpType.max)
                nc.sync.dma_start(
                    out=out[g * P:(g + k) * P, :].rearrange("(k p) c -> p k c", k=k),
                    in_=t1[:, 0:k * P].rearrange("p (k c) -> p k c", c=P),
                )
                g += k

    nc.compile()
    return nc


def make_inputs(x, W, att_src, att_dst, gamma, beta, meta, cfg: Cfg):
    import ml_dtypes
    x = np.asarray(x, np.float32)
    W = np.asarray(W, np.float32)
    att_src = np.asarray(att_src, np.float32)
    att_dst = np.asarray(att_dst, np.float32)
    apat = np.zeros((P, 8), np.float32)
    for h in range(H):
        apat[h * C:(h + 1) * C, h] = att_src[h]
        apat[h * C:(h + 1) * C, 4 + h] = att_dst[h]
    xTb = np.ascontiguousarray(x.T).astype(ml_dtypes.bfloat16)
    wtb = np.ascontiguousarray(W.T).astype(ml_dtypes.bfloat16)
    iota = np.arange(P, dtype=np.float32).reshape(1, P)
    gam = np.asarray(gamma, np.float32).reshape(P, 1)
    bet = np.asarray(beta, np.float32).reshape(P, 1)
    in_maps = []
    for c_ in range(cfg.ncores):
        in_maps.append(
            {
                "xTb": xTb,
                "wtb": wtb,
                "w_of": W,
                "apat": apat,
                "iota_in": iota,
                "iota_col": np.arange(P, dtype=np.float32).reshape(P, 1),
                "gamma_c": gam,
                "beta_c": bet,
                "meta_rel": np.ascontiguousarray(cfg.rel[c_]),
                "metaTb": np.ascontiguousarray(cfg.relT[c_]),
                "idx_in": np.ascontiguousarray(cfg.idxw[c_]),
                "blkbase": np.ascontiguousarray(cfg.blkbase[c_]),
            }
        )
    return in_maps


def assemble_output(core_outs, out_map, cfg: Cfg, n):
    full = np.empty((n, P), np.float32)
    for c_ in range(cfg.ncores):
        for g, (nb_lo, n_valid) in enumerate(out_map[c_]):
            if n_valid > 0:
                full[nb_lo:nb_lo + n_valid] = core_outs[c_][g * P:g * P + n_valid]
    return full


def kernel(**inputs) -> np.ndarray:
    from concourse.bass_utils import run_bass_kernel_spmd

    cfg, _, out_map = preprocess(inputs["edge_index"])
    nc = build_program(cfg)
    in_maps = make_inputs(
        inputs["x"], inputs["W"], inputs["att_src"], inputs["att_dst"],
        inputs["gamma"], inputs["beta"], None, cfg,
    )
    res = run_bass_kernel_spmd(nc, in_maps, core_ids=list(range(NCORES)))
    core_outs = [res.results[c_]["out"] for c_ in range(NCORES)]
    return assemble_output(core_outs, out_map, cfg, N)
